# revision 3
# baseline (speedup 1.0000x reference)
"""Trainium2 (8 NeuronCore) kernel for bilinear pairwise attention:

    out = softmax((Ws @ W[0]) @ Ws.T + b[0], axis=1)     N=4096, D=2048

Sharding: rows of the NxN score matrix are sharded across 8 cores (512
rows each).  The DxD bilinear weight W and the full key matrix Ws.T are
replicated to every core; each core computes and softmaxes its own 512
rows.  No collectives.

All device inputs are cast to fp16 on the host.  fp16 matmuls run at
full TensorE rate (1 col/cycle, cost-model-confirmed) and end-to-end
fp16 rounding lands at rel err ~3.2e-3 (gate 2e-2), same level as the
fp32r path, while read traffic halves to 26 MiB/core (~76 us at 358
GB/s).  The kernel is cleanly compute-bound: 768 matmuls x 215.8 ns
(N=512 warm) = 166 us of PE time, and the traced stream runs them
back-to-back with zero stalls -- key slabs prefetch during stage 1
under the DMA slack.

Math per core c (M = 512 rows):
  stage 1: tT[d, m] = sum_k W[k, d] * WsT_shard[k, m]    (tT = (Ws_c @ W).T)
  stage 2: A[m, j]  = sum_d tT[d, m] * WsT_full[d, j]    (A  = t @ Ws.T)
  softmax over j (b[0] is a constant shift -> softmax-invariant, dropped)

Softmax is chunked per 512-column PSUM bank: -max / exp(x-max) / sum
fused into the PSUM->SBUF eviction (exp results stored fp16).  The
normalization factor is shift-invariant, so the row statistics over
chunks 0..6 are computed with shift S = max(chunks 0..6) in the shadow
of the final matmul groups; only chunk 7's terms are added on the
critical path (with an exp clamp at 1e30 -- exact -- for rows whose
global max lives in chunk 7 by >88).  The last two column chunks are
interleaved per m-tile, doubling each m-tile's epilogue shadow window.
Rescale is split DVE/ACT and stores go out in segments on both HWDGE
rings (a dma_start costs ~0.6 us of issuing-engine time, so per-chunk
stores would serialize on the issue).
"""

import numpy as np

N, D = 4096, 2048
NCORES = 8
M = N // NCORES      # 512 output rows per core
P = 128              # SBUF partitions
KT = D // P          # 16 contraction tiles (stage 1)
DT = D // P          # 16 contraction tiles (stage 2)
MT = M // P          # 4 row tiles per core
JCH = 512            # column chunk = one fp32 PSUM bank
JT = N // JCH        # 8 column chunks
QW = 512             # stage-1 d_out quarter width (4 PSUM banks)
NQ = D // QW         # 4 quarters
WKK = KT // 2        # stage-1 weight chunks per quarter (2 k-tiles each)
GSL = 4              # d-tiles per key-slab DMA (512 KiB fp16)

_NC_CACHE = None


def _build_nc():
    import concourse.tile as tile
    from concourse import bacc, mybir

    f32 = mybir.dt.float32
    f16 = mybir.dt.float16
    bf16 = mybir.dt.bfloat16
    X = mybir.AxisListType.X
    EXP = mybir.ActivationFunctionType.Exp
    ADD = mybir.AluOpType.add
    MIN = mybir.AluOpType.min

    nc = bacc.Bacc("TRN2", target_bir_lowering=False, debug=False)
    # pre-tiled host layouts (see make_in_maps)
    shard = nc.dram_tensor("wsT_shard", [P, KT, M], f16, kind="ExternalInput").ap()
    wmat = nc.dram_tensor("w_mat", [NQ, WKK, P, 2, QW], f16, kind="ExternalInput").ap()
    wst = nc.dram_tensor(
        "wsT_full", [JT, DT // GSL, P, GSL, JCH], f16, kind="ExternalInput"
    ).ap()
    out = nc.dram_tensor("out", [M, N], f16, kind="ExternalOutput").ap()

    with tile.TileContext(nc) as tc:
        with (
            tc.tile_pool(name="singles", bufs=1) as singles,
            tc.tile_pool(name="wq", bufs=10) as wpool,
            tc.tile_pool(name="wstp", bufs=12) as wstpool,
            tc.tile_pool(name="stats", bufs=1) as stats,
            tc.tile_pool(name="psum", bufs=8, space="PSUM") as psum,
        ):
            # --- query shard (16 x [128, 512]); chunks are interleaved
            # with the first W chunks below so the PE can start stage 1
            # as early as possible
            shard_sb = singles.tile([P, KT, M], f16, name="shard_sb")

            def load_shard_chunk(kk):
                lo = 2 * kk + 2
                nc.sync.dma_start(
                    out=shard_sb[:, lo : lo + 2, :], in_=shard[:, lo : lo + 2, :]
                )

            nc.sync.dma_start(out=shard_sb[:, 0:2, :], in_=shard[:, 0:2, :])

            # --- stage 1: tT[d, m], d_out processed in 4 quarters of 512
            tT = singles.tile([P, DT, M], f16, name="tT")
            for q in range(NQ):
                ps1 = [
                    psum.tile([P, JCH], f32, name=f"ps1_{q}_{i}", tag="ps")
                    for i in range(4)
                ]
                for kk in range(WKK):
                    wq_t = wpool.tile([P, 2, QW], f16, name="wq_t")
                    nc.sync.dma_start(out=wq_t, in_=wmat[q, kk])
                    if q == 0 and kk < WKK - 1:
                        load_shard_chunk(kk)
                    for ki in range(2):
                        for i in range(4):
                            nc.tensor.matmul(
                                ps1[i],
                                wq_t[:, ki, i * P : (i + 1) * P],
                                shard_sb[:, kk * 2 + ki, :],
                                start=(kk == 0 and ki == 0),
                                stop=(kk == WKK - 1 and ki == 1),
                            )
                for i in range(4):
                    nc.vector.tensor_copy(out=tT[:, q * 4 + i, :], in_=ps1[i])

            # --- stage 2 + chunked softmax stats (exp results in fp16)
            a_tiles = [singles.tile([P, N], f16, name=f"a{m}") for m in range(MT)]
            ncmax = [stats.tile([P, JT], f32, name=f"ncmax{m}") for m in range(MT)]
            csum = [stats.tile([P, JT], f32, name=f"csum{m}") for m in range(MT)]
            ngm = [stats.tile([P, 1], f32, name=f"ngm{m}") for m in range(MT)]
            sfac = [stats.tile([P, JT], f32, name=f"sfac{m}") for m in range(MT)]
            wsum = [stats.tile([P, JT], f32, name=f"wsum{m}") for m in range(MT)]
            rsum6 = [stats.tile([P, 1], f32, name=f"rsum6{m}") for m in range(MT)]
            rsum = [stats.tile([P, 1], f32, name=f"rsum{m}") for m in range(MT)]
            rinv = [stats.tile([P, 1], f32, name=f"rinv{m}") for m in range(MT)]
            factor = [stats.tile([P, JT], f32, name=f"factor{m}") for m in range(MT)]

            COPY = mybir.ActivationFunctionType.Copy

            def epilogue_b(m):
                """Finish softmax stats for m-tile m (chunk 7 terms only),
                rescale split across DVE/ACT/GpSimd, store 3 segments."""
                # sfac_7 on ACT right behind this m's chunk-7 exp
                nc.scalar.activation(
                    out=sfac[m][:, JT - 1 : JT],
                    in_=ncmax[m][:, JT - 1 : JT],
                    func=EXP,
                    bias=ngm[m],
                    scale=-1.0,
                )
                # If chunk 7 holds the row's global max by more than ~88,
                # exp overflows to inf and factor_7 becomes inf*0=NaN.
                # Clamping is exact in that regime: factor_7 -> 1/csum_7
                # and the other chunks' factors underflow to 0 as they
                # mathematically should.
                nc.vector.tensor_scalar_min(
                    sfac[m][:, JT - 1 : JT], sfac[m][:, JT - 1 : JT], 1.0e30
                )
                nc.vector.tensor_mul(
                    out=wsum[m][:, JT - 1 : JT],
                    in0=sfac[m][:, JT - 1 : JT],
                    in1=csum[m][:, JT - 1 : JT],
                )
                nc.vector.tensor_add(
                    out=rsum[m], in0=rsum6[m], in1=wsum[m][:, JT - 1 : JT]
                )
                nc.vector.reciprocal(out=rinv[m], in_=rsum[m])
                nc.vector.tensor_scalar_mul(factor[m], sfac[m], rinv[m])
                # rescale chunks 0-4,7 on DVE (~0.35us each), 5-6 on ACT
                # via Copy with per-partition scale (~0.8us each); the two
                # engines run concurrently.  (GpSimd tensor_scalar measured
                # 7.5us/chunk and stalls concurrent DVE ops -- unusable.)
                # Stores are grouped into segments (a dma_start costs
                # ~0.6us of issuing-engine time, so per-chunk stores
                # serialize on the issue, not the transfer): {0-2},{3,4},
                # {7} on the SP ring, {5,6} on the ACT ring.
                row = out[m * P : (m + 1) * P, :]
                for j2 in (0, 1, 2, 3, 4, 7):
                    a_sl = a_tiles[m][:, j2 * JCH : (j2 + 1) * JCH]
                    nc.vector.tensor_scalar_mul(a_sl, a_sl, factor[m][:, j2 : j2 + 1])
                    if j2 == 2:
                        nc.sync.dma_start(
                            out=row[:, 0 : 3 * JCH], in_=a_tiles[m][:, 0 : 3 * JCH]
                        )
                    elif j2 == 4:
                        nc.sync.dma_start(
                            out=row[:, 3 * JCH : 5 * JCH],
                            in_=a_tiles[m][:, 3 * JCH : 5 * JCH],
                        )
                    elif j2 == 7:
                        nc.sync.dma_start(
                            out=row[:, 7 * JCH : 8 * JCH],
                            in_=a_tiles[m][:, 7 * JCH : 8 * JCH],
                        )
                for j2 in (5, 6):
                    a_sl = a_tiles[m][:, j2 * JCH : (j2 + 1) * JCH]
                    nc.scalar.activation(
                        out=a_sl, in_=a_sl, func=COPY, bias=0.0,
                        scale=factor[m][:, j2 : j2 + 1],
                    )
                nc.scalar.dma_start(
                    out=row[:, 5 * JCH : 7 * JCH], in_=a_tiles[m][:, 5 * JCH : 7 * JCH]
                )

            def load_slab(j):
                slabs = []
                for g in range(DT // GSL):
                    wst_sl = wstpool.tile([P, GSL, JCH], f16, name="wst_sl")
                    if j < 2:
                        # write-before-write gate: orders the slab DMA after
                        # stage-1 q1/q2 so the key-slab prefetch doesn't
                        # steal HBM bandwidth from the W feed
                        nc.vector.tensor_copy(
                            out=wst_sl[:, 0, 0:1], in_=tT[:, 4 * (j + 1), 0:1]
                        )
                    nc.sync.dma_start(out=wst_sl, in_=wst[j, g])
                    slabs.append(wst_sl)
                return slabs

            def chunk_group(j, m, slabs):
                ps2 = psum.tile([P, JCH], f32, name="ps2", tag="ps")
                for d in range(DT):
                    nc.tensor.matmul(
                        ps2,
                        tT[:, d, m * P : (m + 1) * P],
                        slabs[d // GSL][:, d % GSL, :],
                        start=(d == 0),
                        stop=(d == DT - 1),
                    )
                # chunk softmax: -max, then exp(x - max) with running sum
                nc.vector.reduce_max(
                    out=ncmax[m][:, j : j + 1], in_=ps2, axis=X, negate=True
                )
                nc.scalar.activation(
                    out=a_tiles[m][:, j * JCH : (j + 1) * JCH],
                    in_=ps2,
                    func=EXP,
                    bias=ncmax[m][:, j : j + 1],
                    scale=1.0,
                    accum_out=csum[m][:, j : j + 1],
                )

            def part_a(m):
                # stats over chunks 0..6, run in the shadow of the final
                # matmul groups.  The softmax factor is invariant to the
                # shift S, so S = max over the first 7 chunks works; only
                # chunk 7's contribution is added late in epilogue_b.
                nc.vector.tensor_reduce(
                    out=ngm[m], in_=ncmax[m][:, 0 : JT - 1], axis=X, op=MIN
                )
                nc.scalar.activation(
                    out=sfac[m][:, 0 : JT - 1],
                    in_=ncmax[m][:, 0 : JT - 1],
                    func=EXP,
                    bias=ngm[m],
                    scale=-1.0,
                )
                nc.vector.tensor_mul(
                    out=wsum[m][:, 0 : JT - 1],
                    in0=sfac[m][:, 0 : JT - 1],
                    in1=csum[m][:, 0 : JT - 1],
                )
                nc.vector.tensor_reduce(
                    out=rsum6[m], in_=wsum[m][:, 0 : JT - 1], axis=X, op=ADD
                )

            for j in range(JT - 2):
                slabs = load_slab(j)
                for m in range(MT):
                    chunk_group(j, m, slabs)

            # Final two column chunks are interleaved per m-tile:
            # (j6,m0),(j7,m0),(j6,m1),... so each m-tile's softmax epilogue
            # gets a 6.8us shadow window under the remaining matmul groups
            # instead of 3.4us, and the last m-tile's epilogue chain starts
            # right at the final matmul.
            slabs6 = load_slab(JT - 2)
            slabs7 = load_slab(JT - 1)
            for m in range(MT):
                chunk_group(JT - 2, m, slabs6)
                part_a(m)
                chunk_group(JT - 1, m, slabs7)
                epilogue_b(m)

    nc.compile()
    return nc


def get_nc():
    global _NC_CACHE
    if _NC_CACHE is None:
        _NC_CACHE = _build_nc()
    return _NC_CACHE


def make_in_maps(Ws, W):
    Ws = np.asarray(Ws, dtype=np.float32)
    W0 = np.asarray(W, dtype=np.float32).reshape(D, D)
    # W pre-tile: [q, kk, p, ki, c] so each [128, 2, 512] chunk is a
    # contiguous 2 KB/partition read
    w_t = np.ascontiguousarray(
        W0.reshape(WKK, 2, P, NQ, QW).transpose(3, 0, 2, 1, 4)
    ).astype(np.float16)
    # Ws.T pre-tile: [j, g, p, ti, c] so each [128, 4, 512] slab is a
    # contiguous 4 KB/partition read
    WsT = np.ascontiguousarray(Ws.T)  # [D, N]
    wst_t = np.ascontiguousarray(
        WsT.reshape(DT // GSL, GSL, P, JT, JCH).transpose(3, 0, 2, 1, 4)
    ).astype(np.float16)
    in_maps = []
    for c in range(NCORES):
        shard_t = np.ascontiguousarray(
            Ws[c * M : (c + 1) * M, :].T.reshape(KT, P, M).transpose(1, 0, 2)
        ).astype(np.float16)
        in_maps.append({"wsT_shard": shard_t, "w_mat": w_t, "wsT_full": wst_t})
    return in_maps


def _run_device(in_maps):
    from concourse.bass_utils import run_bass_kernel_spmd

    nc = get_nc()
    res = run_bass_kernel_spmd(nc, in_maps, core_ids=list(range(NCORES)))
    return np.concatenate(
        [res.results[c]["out"] for c in range(NCORES)], axis=0
    )


def kernel(Ws, W, b, **_unused):
    # b[0] is a constant additive shift on every score; softmax over
    # axis=1 is invariant to it, so it never enters the device kernel.
    in_maps = make_in_maps(Ws, W)
    try:
        out = _run_device(in_maps)
    except Exception as e:  # transient device failures recover on retry
        import sys, traceback

        traceback.print_exc()
        print(f"device run failed ({e!r}); retrying once", file=sys.stderr)
        try:
            out = _run_device(in_maps)
        except Exception:
            traceback.print_exc()
            print("device retry failed; numpy fallback", file=sys.stderr)
            Wsf = np.asarray(Ws, dtype=np.float32)
            A = (Wsf @ np.asarray(W, np.float32).reshape(D, D)) @ Wsf.T
            A += np.asarray(b, np.float32).reshape(-1)[0]
            A -= A.max(axis=1, keepdims=True)
            np.exp(A, out=A)
            A /= A.sum(axis=1, keepdims=True)
            return A
    return np.ascontiguousarray(out.astype(np.float32))


if __name__ == "__main__":
    rng = np.random.default_rng(0)
    Ws = rng.standard_normal((N, D), dtype=np.float32)
    W = (rng.standard_normal((1, D, D)) / np.sqrt(D)).astype(np.float32)
    b = np.zeros((1,), dtype=np.float32)
    res = kernel(Ws=Ws, W=W, b=b)
    print(res.shape, res.dtype, res.sum())


# revision 5
# speedup vs baseline: 1.0026x; 1.0026x over previous
"""Trainium2 (8 NeuronCore) kernel for bilinear pairwise attention:

    out = softmax((Ws @ W[0]) @ Ws.T + b[0], axis=1)     N=4096, D=2048

Sharding: rows of the NxN score matrix are sharded across 8 cores (512
rows each).  The DxD bilinear weight W and the full key matrix Ws.T are
replicated to every core; each core computes and softmaxes its own 512
rows.

All device inputs are cast to fp16 on the host.  fp16 matmuls run at
full TensorE rate (1 col/cycle, cost-model-confirmed) and end-to-end
fp16 rounding lands at rel err ~3.2e-3 (gate 2e-2), while read traffic
halves to 26 MiB/core (~76 us at 358 GB/s).  The kernel is cleanly
compute-bound: the 832-matmul stream runs back-to-back at the 215.8 ns
(N=512, warm) machine issue rate with zero stalls -- key slabs
prefetch during stage 1 under the DMA slack.

Math per core c (M = 512 rows):
  stage 1: tT[d, m] = sum_k W[k, d] * WsT_shard[k, m]    (tT = (Ws_c @ W).T)
  stage 2: A[m, j]  = sum_d tT[d, m] * WsT_full[d, j]    (A  = t @ Ws.T)
  softmax over j (b[0] is a constant shift -> softmax-invariant, dropped)

Softmax is chunked per PSUM bank (7x512 columns plus two final
256-wide groups): -max / exp(x-max) / sum fused into the PSUM->SBUF
eviction (exp results stored fp16).  The normalization factor is
shift-invariant, so the row statistics over the first 8 groups are
computed with shift S = max(those groups) in the shadow of the final
matmul groups; only the last 256-wide group's terms are added on the
critical path (with an exp clamp at 1e30 -- exact -- for rows whose
global max lives there by >88).  The final three groups are
interleaved per m-tile, keeping a ~6.8 us epilogue shadow window, and
the half-width last group halves the post-matmul reduce/exp chain.
Rescale is split DVE/ACT and stores go out in segments on both HWDGE
rings (a dma_start costs ~0.6 us of issuing-engine time, so per-chunk
stores would serialize on the issue).
"""

import numpy as np

N, D = 4096, 2048
NCORES = 8
M = N // NCORES      # 512 output rows per core
P = 128              # SBUF partitions
KT = D // P          # 16 contraction tiles (stage 1)
DT = D // P          # 16 contraction tiles (stage 2)
MT = M // P          # 4 row tiles per core
JCH = 512            # column chunk = one fp32 PSUM bank
JT = N // JCH        # 8 column chunks
QW = 512             # stage-1 d_out quarter width (4 PSUM banks)
NQ = D // QW         # 4 quarters
WKK = KT // 2        # stage-1 weight chunks per quarter (2 k-tiles each)
GSL = 4              # d-tiles per key-slab DMA (512 KiB fp16)

_NC_CACHE = None


def _build_nc():
    import concourse.tile as tile
    from concourse import bacc, mybir

    f32 = mybir.dt.float32
    f16 = mybir.dt.float16
    bf16 = mybir.dt.bfloat16
    X = mybir.AxisListType.X
    EXP = mybir.ActivationFunctionType.Exp
    ADD = mybir.AluOpType.add
    MIN = mybir.AluOpType.min

    nc = bacc.Bacc("TRN2", target_bir_lowering=False, debug=False)
    # pre-tiled host layouts (see make_in_maps)
    shard = nc.dram_tensor("wsT_shard", [P, KT, M], f16, kind="ExternalInput").ap()
    wmat = nc.dram_tensor("w_mat", [NQ, WKK, P, 2, QW], f16, kind="ExternalInput").ap()
    wst = nc.dram_tensor(
        "wsT_full", [JT, DT // GSL, P, GSL, JCH], f16, kind="ExternalInput"
    ).ap()
    out = nc.dram_tensor("out", [M, N], f16, kind="ExternalOutput").ap()

    with tile.TileContext(nc) as tc:
        with (
            tc.tile_pool(name="singles", bufs=1) as singles,
            tc.tile_pool(name="wq", bufs=10) as wpool,
            tc.tile_pool(name="wstp", bufs=12) as wstpool,
            tc.tile_pool(name="stats", bufs=1) as stats,
            tc.tile_pool(name="psum", bufs=8, space="PSUM") as psum,
        ):
            # --- query shard (16 x [128, 512]); chunks are interleaved
            # with the first W chunks below so the PE can start stage 1
            # as early as possible
            shard_sb = singles.tile([P, KT, M], f16, name="shard_sb")

            def load_shard_chunk(kk):
                lo = 2 * kk + 2
                nc.sync.dma_start(
                    out=shard_sb[:, lo : lo + 2, :], in_=shard[:, lo : lo + 2, :]
                )

            nc.sync.dma_start(out=shard_sb[:, 0:2, :], in_=shard[:, 0:2, :])

            # --- stage 1: tT[d, m], d_out processed in 4 quarters of 512
            tT = singles.tile([P, DT, M], f16, name="tT")
            for q in range(NQ):
                ps1 = [
                    psum.tile([P, JCH], f32, name=f"ps1_{q}_{i}", tag="ps")
                    for i in range(4)
                ]
                for kk in range(WKK):
                    wq_t = wpool.tile([P, 2, QW], f16, name="wq_t")
                    nc.sync.dma_start(out=wq_t, in_=wmat[q, kk])
                    if q == 0 and kk < WKK - 1:
                        load_shard_chunk(kk)
                    for ki in range(2):
                        for i in range(4):
                            nc.tensor.matmul(
                                ps1[i],
                                wq_t[:, ki, i * P : (i + 1) * P],
                                shard_sb[:, kk * 2 + ki, :],
                                start=(kk == 0 and ki == 0),
                                stop=(kk == WKK - 1 and ki == 1),
                            )
                for i in range(4):
                    nc.vector.tensor_copy(out=tT[:, q * 4 + i, :], in_=ps1[i])

            # --- stage 2 + chunked softmax stats (exp results in fp16)
            # Column chunks: 7 of 512 plus two final 256-wide groups (7a,
            # 7b) so the last group's serial reduce/exp/rescale/store
            # chain after the final matmul is half-width.
            NST = JT + 1        # 9 stat columns
            CHUNKS = [(i, i * JCH, JCH) for i in range(JT - 1)] + [
                (JT - 1, (JT - 1) * JCH, JCH // 2),
                (JT, (JT - 1) * JCH + JCH // 2, JCH // 2),
            ]  # (stat idx, col offset, width)
            a_tiles = [singles.tile([P, N], f16, name=f"a{m}") for m in range(MT)]
            ncmax = [stats.tile([P, NST], f32, name=f"ncmax{m}") for m in range(MT)]
            csum = [stats.tile([P, NST], f32, name=f"csum{m}") for m in range(MT)]
            ngm = [stats.tile([P, 1], f32, name=f"ngm{m}") for m in range(MT)]
            sfac = [stats.tile([P, NST], f32, name=f"sfac{m}") for m in range(MT)]
            wsum = [stats.tile([P, NST], f32, name=f"wsum{m}") for m in range(MT)]
            rsum6 = [stats.tile([P, 1], f32, name=f"rsum6{m}") for m in range(MT)]
            rsum = [stats.tile([P, 1], f32, name=f"rsum{m}") for m in range(MT)]
            rinv = [stats.tile([P, 1], f32, name=f"rinv{m}") for m in range(MT)]
            factor = [stats.tile([P, NST], f32, name=f"factor{m}") for m in range(MT)]

            COPY = mybir.ActivationFunctionType.Copy

            def epilogue_b(m):
                """Finish softmax stats for m-tile m (group 7b terms only),
                rescale split across DVE/ACT, store segments on both rings."""
                H = JCH // 2
                # sfac_7b on ACT right behind this m's group-7b exp
                nc.scalar.activation(
                    out=sfac[m][:, NST - 1 : NST],
                    in_=ncmax[m][:, NST - 1 : NST],
                    func=EXP,
                    bias=ngm[m],
                    scale=-1.0,
                )
                # If group 7b holds the row's global max by more than ~88,
                # exp overflows to inf and its factor becomes inf*0=NaN.
                # Clamping is exact in that regime: the factor -> 1/csum
                # and the other chunks' factors underflow to 0 as they
                # mathematically should.
                nc.vector.tensor_scalar_min(
                    sfac[m][:, NST - 1 : NST], sfac[m][:, NST - 1 : NST], 1.0e30
                )
                # rsum = csum_7b * sfac_7b + rsum(0..7a) in one DVE op
                nc.vector.scalar_tensor_tensor(
                    out=rsum[m],
                    in0=csum[m][:, NST - 1 : NST],
                    scalar=sfac[m][:, NST - 1 : NST],
                    in1=rsum6[m],
                    op0=mybir.AluOpType.mult,
                    op1=ADD,
                )
                nc.vector.reciprocal(out=rinv[m], in_=rsum[m])
                nc.vector.tensor_scalar_mul(factor[m], sfac[m], rinv[m])
                # rescale: DVE takes 7b then chunks 0-4 (~0.35us each at
                # 512 wide); ACT takes 7a then 5,6 via Copy with
                # per-partition scale (~0.8us each).  (GpSimd tensor_scalar
                # measured 7.5us/chunk and stalls concurrent DVE ops --
                # unusable.)  Stores are segments (a dma_start costs
                # ~0.6us of issuing-engine time): the freshly-finished
                # tail cols 3584:4096 go out first on SP, then {0-2},
                # {3,4} on SP while {5,6} rides the ACT ring.
                row = out[m * P : (m + 1) * P, :]
                c7 = (JT - 1) * JCH
                # DVE: group 7b first
                a_sl = a_tiles[m][:, c7 + H : c7 + 2 * H]
                nc.vector.tensor_scalar_mul(
                    a_sl, a_sl, factor[m][:, NST - 1 : NST]
                )
                # ACT: group 7a first, then chunks 5, 6
                a_sl = a_tiles[m][:, c7 : c7 + H]
                nc.scalar.activation(
                    out=a_sl, in_=a_sl, func=COPY, bias=0.0,
                    scale=factor[m][:, JT - 1 : JT],
                )
                # tail cols (both halves of old chunk 7) on SP ring
                nc.sync.dma_start(
                    out=row[:, c7 : c7 + 2 * H], in_=a_tiles[m][:, c7 : c7 + 2 * H]
                )
                for j2 in (0, 1, 2, 3, 4):
                    a_sl = a_tiles[m][:, j2 * JCH : (j2 + 1) * JCH]
                    nc.vector.tensor_scalar_mul(a_sl, a_sl, factor[m][:, j2 : j2 + 1])
                    if j2 == 2:
                        nc.sync.dma_start(
                            out=row[:, 0 : 3 * JCH], in_=a_tiles[m][:, 0 : 3 * JCH]
                        )
                    elif j2 == 4:
                        nc.sync.dma_start(
                            out=row[:, 3 * JCH : 5 * JCH],
                            in_=a_tiles[m][:, 3 * JCH : 5 * JCH],
                        )
                for j2 in (5, 6):
                    a_sl = a_tiles[m][:, j2 * JCH : (j2 + 1) * JCH]
                    nc.scalar.activation(
                        out=a_sl, in_=a_sl, func=COPY, bias=0.0,
                        scale=factor[m][:, j2 : j2 + 1],
                    )
                nc.scalar.dma_start(
                    out=row[:, 5 * JCH : 7 * JCH], in_=a_tiles[m][:, 5 * JCH : 7 * JCH]
                )

            def load_slab(j):
                slabs = []
                for g in range(DT // GSL):
                    wst_sl = wstpool.tile([P, GSL, JCH], f16, name="wst_sl")
                    if j < 2:
                        # write-before-write gate: orders the slab DMA after
                        # stage-1 q1/q2 so the key-slab prefetch doesn't
                        # steal HBM bandwidth from the W feed
                        nc.vector.tensor_copy(
                            out=wst_sl[:, 0, 0:1], in_=tT[:, 4 * (j + 1), 0:1]
                        )
                    nc.sync.dma_start(out=wst_sl, in_=wst[j, g])
                    slabs.append(wst_sl)
                return slabs

            def chunk_group(si, m, slabs, c0, width, s0):
                ps2 = psum.tile([P, width], f32, name="ps2", tag="ps")
                for d in range(DT):
                    nc.tensor.matmul(
                        ps2,
                        tT[:, d, m * P : (m + 1) * P],
                        slabs[d // GSL][:, d % GSL, s0 : s0 + width],
                        start=(d == 0),
                        stop=(d == DT - 1),
                    )
                # chunk softmax: -max, then exp(x - max) with running sum
                nc.vector.reduce_max(
                    out=ncmax[m][:, si : si + 1], in_=ps2, axis=X, negate=True
                )
                nc.scalar.activation(
                    out=a_tiles[m][:, c0 : c0 + width],
                    in_=ps2,
                    func=EXP,
                    bias=ncmax[m][:, si : si + 1],
                    scale=1.0,
                    accum_out=csum[m][:, si : si + 1],
                )

            def part_a(m):
                # stats over chunk groups 0..7a, run in the shadow of the
                # final matmul groups.  The softmax factor is invariant to
                # the shift S, so S = max over the first 8 groups works;
                # only group 7b's contribution is added late in epilogue_b.
                nc.vector.tensor_reduce(
                    out=ngm[m], in_=ncmax[m][:, 0 : NST - 1], axis=X, op=MIN
                )
                nc.scalar.activation(
                    out=sfac[m][:, 0 : NST - 1],
                    in_=ncmax[m][:, 0 : NST - 1],
                    func=EXP,
                    bias=ngm[m],
                    scale=-1.0,
                )
                nc.vector.tensor_mul(
                    out=wsum[m][:, 0 : NST - 1],
                    in0=sfac[m][:, 0 : NST - 1],
                    in1=csum[m][:, 0 : NST - 1],
                )
                nc.vector.tensor_reduce(
                    out=rsum6[m], in_=wsum[m][:, 0 : NST - 1], axis=X, op=ADD
                )

            for si, c0, width in CHUNKS[: JT - 2]:
                slabs = load_slab(si)
                for m in range(MT):
                    chunk_group(si, m, slabs, c0, width, 0)

            # The final three chunk groups (j6, 7a, 7b) are interleaved
            # per m-tile so each m-tile's softmax epilogue gets a ~6.8us
            # shadow window under the remaining matmul groups, and the
            # last m-tile's epilogue chain starts right at the final
            # (half-width) matmul group.
            slabs6 = load_slab(JT - 2)
            slabs7 = load_slab(JT - 1)
            for m in range(MT):
                chunk_group(JT - 2, m, slabs6, (JT - 2) * JCH, JCH, 0)
                chunk_group(JT - 1, m, slabs7, (JT - 1) * JCH, JCH // 2, 0)
                part_a(m)
                chunk_group(
                    JT, m, slabs7, (JT - 1) * JCH + JCH // 2, JCH // 2, JCH // 2
                )
                epilogue_b(m)

    nc.compile()
    return nc


def get_nc():
    global _NC_CACHE
    if _NC_CACHE is None:
        _NC_CACHE = _build_nc()
    return _NC_CACHE


def make_in_maps(Ws, W):
    Ws = np.asarray(Ws, dtype=np.float32)
    W0 = np.asarray(W, dtype=np.float32).reshape(D, D)
    # W pre-tile: [q, kk, p, ki, c] so each [128, 2, 512] chunk is a
    # contiguous 2 KB/partition read
    w_t = np.ascontiguousarray(
        W0.reshape(WKK, 2, P, NQ, QW).transpose(3, 0, 2, 1, 4)
    ).astype(np.float16)
    # Ws.T pre-tile: [j, g, p, ti, c] so each [128, 4, 512] slab is a
    # contiguous 4 KB/partition read
    WsT = np.ascontiguousarray(Ws.T)  # [D, N]
    wst_t = np.ascontiguousarray(
        WsT.reshape(DT // GSL, GSL, P, JT, JCH).transpose(3, 0, 2, 1, 4)
    ).astype(np.float16)
    in_maps = []
    for c in range(NCORES):
        shard_t = np.ascontiguousarray(
            Ws[c * M : (c + 1) * M, :].T.reshape(KT, P, M).transpose(1, 0, 2)
        ).astype(np.float16)
        in_maps.append({"wsT_shard": shard_t, "w_mat": w_t, "wsT_full": wst_t})
    return in_maps


def _run_device(in_maps):
    from concourse.bass_utils import run_bass_kernel_spmd

    nc = get_nc()
    res = run_bass_kernel_spmd(nc, in_maps, core_ids=list(range(NCORES)))
    return np.concatenate(
        [res.results[c]["out"] for c in range(NCORES)], axis=0
    )


def kernel(Ws, W, b, **_unused):
    # b[0] is a constant additive shift on every score; softmax over
    # axis=1 is invariant to it, so it never enters the device kernel.
    in_maps = make_in_maps(Ws, W)
    try:
        out = _run_device(in_maps)
    except Exception as e:  # transient device failures recover on retry
        import sys, traceback

        traceback.print_exc()
        print(f"device run failed ({e!r}); retrying once", file=sys.stderr)
        try:
            out = _run_device(in_maps)
        except Exception:
            traceback.print_exc()
            print("device retry failed; numpy fallback", file=sys.stderr)
            Wsf = np.asarray(Ws, dtype=np.float32)
            A = (Wsf @ np.asarray(W, np.float32).reshape(D, D)) @ Wsf.T
            A += np.asarray(b, np.float32).reshape(-1)[0]
            A -= A.max(axis=1, keepdims=True)
            np.exp(A, out=A)
            A /= A.sum(axis=1, keepdims=True)
            return A
    return np.ascontiguousarray(out.astype(np.float32))


if __name__ == "__main__":
    rng = np.random.default_rng(0)
    Ws = rng.standard_normal((N, D), dtype=np.float32)
    W = (rng.standard_normal((1, D, D)) / np.sqrt(D)).astype(np.float32)
    b = np.zeros((1,), dtype=np.float32)
    res = kernel(Ws=Ws, W=W, b=b)
    print(res.shape, res.dtype, res.sum())


# revision 6
# speedup vs baseline: 1.0040x; 1.0014x over previous
"""Trainium2 (8 NeuronCore) kernel for bilinear pairwise attention:

    out = softmax((Ws @ W[0]) @ Ws.T + b[0], axis=1)     N=4096, D=2048

Sharding: rows of the NxN score matrix are sharded across 8 cores (512
rows each).  The DxD bilinear weight W and the full key matrix Ws.T are
replicated to every core; each core computes and softmaxes its own 512
rows.

All device inputs are cast to fp16 on the host.  fp16 matmuls run at
full TensorE rate (1 col/cycle, cost-model-confirmed) and end-to-end
fp16 rounding lands at rel err ~3.2e-3 (gate 2e-2), while read traffic
halves to 26 MiB/core (~76 us at 358 GB/s).  The kernel is cleanly
compute-bound: the 832-matmul stream runs back-to-back at the 215.8 ns
(N=512, warm) machine issue rate with zero stalls -- key slabs
prefetch during stage 1 under the DMA slack.

Math per core c (M = 512 rows):
  stage 1: tT[d, m] = sum_k W[k, d] * WsT_shard[k, m]    (tT = (Ws_c @ W).T)
  stage 2: A[m, j]  = sum_d tT[d, m] * WsT_full[d, j]    (A  = t @ Ws.T)
  softmax over j (b[0] is a constant shift -> softmax-invariant, dropped)

Softmax is chunked per PSUM bank (7x512 columns plus two final
256-wide groups): -max / exp(x-max) / sum fused into the PSUM->SBUF
eviction (exp results stored fp16).  The normalization factor is
shift-invariant, so the row statistics over the first 8 groups are
computed with shift S = max(those groups) in the shadow of the final
matmul groups; only the last 256-wide group's terms are added on the
critical path (with an exp clamp at 1e30 -- exact -- for rows whose
global max lives there by >88).  The final three groups are
interleaved per m-tile, keeping a ~6.8 us epilogue shadow window, and
the half-width last group halves the post-matmul reduce/exp chain.
Rescale is split DVE/ACT and stores go out in segments on both HWDGE
rings (a dma_start costs ~0.6 us of issuing-engine time, so per-chunk
stores would serialize on the issue).
"""

import numpy as np

N, D = 4096, 2048
NCORES = 8
M = N // NCORES      # 512 output rows per core
P = 128              # SBUF partitions
KT = D // P          # 16 contraction tiles (stage 1)
DT = D // P          # 16 contraction tiles (stage 2)
MT = M // P          # 4 row tiles per core
JCH = 512            # column chunk = one fp32 PSUM bank
JT = N // JCH        # 8 column chunks
QW = 512             # stage-1 d_out quarter width (4 PSUM banks)
NQ = D // QW         # 4 quarters
WKK = KT // 2        # stage-1 weight chunks per quarter (2 k-tiles each)
GSL = 4              # d-tiles per key-slab DMA (512 KiB fp16)

_NC_CACHE = None


def _build_nc():
    import concourse.tile as tile
    from concourse import bacc, mybir

    f32 = mybir.dt.float32
    f16 = mybir.dt.float16
    bf16 = mybir.dt.bfloat16
    X = mybir.AxisListType.X
    EXP = mybir.ActivationFunctionType.Exp
    ADD = mybir.AluOpType.add
    MIN = mybir.AluOpType.min

    nc = bacc.Bacc("TRN2", target_bir_lowering=False, debug=False)
    # pre-tiled host layouts (see make_in_maps)
    shard = nc.dram_tensor("wsT_shard", [P, KT, M], f16, kind="ExternalInput").ap()
    wmat = nc.dram_tensor("w_mat", [NQ, WKK, P, 2, QW], f16, kind="ExternalInput").ap()
    wst = nc.dram_tensor(
        "wsT_full", [JT, DT // GSL, P, GSL, JCH], f16, kind="ExternalInput"
    ).ap()
    out = nc.dram_tensor("out", [M, N], f16, kind="ExternalOutput").ap()

    with tile.TileContext(nc) as tc:
        with (
            tc.tile_pool(name="singles", bufs=1) as singles,
            tc.tile_pool(name="wq", bufs=10) as wpool,
            tc.tile_pool(name="wstp", bufs=12) as wstpool,
            tc.tile_pool(name="stats", bufs=1) as stats,
            tc.tile_pool(name="psum", bufs=8, space="PSUM") as psum,
        ):
            # --- query shard (16 x [128, 512]); chunks are interleaved
            # with the first W chunks below so the PE can start stage 1
            # as early as possible
            shard_sb = singles.tile([P, KT, M], f16, name="shard_sb")

            def load_shard_chunk(kk):
                lo = 2 * kk + 2
                nc.sync.dma_start(
                    out=shard_sb[:, lo : lo + 2, :], in_=shard[:, lo : lo + 2, :]
                )

            nc.sync.dma_start(out=shard_sb[:, 0:2, :], in_=shard[:, 0:2, :])

            # --- PE warm-up: a DVE memset is the earliest possible SBUF
            # write after the engine preamble (~6.1us); 18 short matmuls
            # on it keep the PE busy from ~7.7us so the HAM un-throttles
            # (needs ~3.4us of sustained activity) right as the first
            # real operands land (~11us) -- stage 1 then runs at 2.4 GHz
            # from roughly its first matmul instead of paying ~2.3us of
            # half-clock on its first ~13 matmuls.  Kept short (256-wide)
            # so a late memset cannot push the real matmul stream back by
            # more than a fraction of the expected gain.
            scratch = singles.tile([P, 2 * P], bf16, name="scratch")
            nc.vector.memset(scratch, 0.0)
            warm = psum.tile([P, 2 * P], f32, name="warm", tag="ps")
            for _ in range(18):
                nc.tensor.matmul(
                    warm, scratch[:, 0:P], scratch, start=True, stop=True
                )

            # --- stage 1: tT[d, m], d_out processed in 4 quarters of 512
            tT = singles.tile([P, DT, M], f16, name="tT")
            for q in range(NQ):
                ps1 = [
                    psum.tile([P, JCH], f32, name=f"ps1_{q}_{i}", tag="ps")
                    for i in range(4)
                ]
                for kk in range(WKK):
                    wq_t = wpool.tile([P, 2, QW], f16, name="wq_t")
                    nc.sync.dma_start(out=wq_t, in_=wmat[q, kk])
                    if q == 0 and kk < WKK - 1:
                        load_shard_chunk(kk)
                    for ki in range(2):
                        for i in range(4):
                            nc.tensor.matmul(
                                ps1[i],
                                wq_t[:, ki, i * P : (i + 1) * P],
                                shard_sb[:, kk * 2 + ki, :],
                                start=(kk == 0 and ki == 0),
                                stop=(kk == WKK - 1 and ki == 1),
                            )
                for i in range(4):
                    nc.vector.tensor_copy(out=tT[:, q * 4 + i, :], in_=ps1[i])

            # --- stage 2 + chunked softmax stats (exp results in fp16)
            # Column chunks: 7 of 512 plus two final 256-wide groups (7a,
            # 7b) so the last group's serial reduce/exp/rescale/store
            # chain after the final matmul is half-width.
            NST = JT + 1        # 9 stat columns
            CHUNKS = [(i, i * JCH, JCH) for i in range(JT - 1)] + [
                (JT - 1, (JT - 1) * JCH, JCH // 2),
                (JT, (JT - 1) * JCH + JCH // 2, JCH // 2),
            ]  # (stat idx, col offset, width)
            a_tiles = [singles.tile([P, N], f16, name=f"a{m}") for m in range(MT)]
            ncmax = [stats.tile([P, NST], f32, name=f"ncmax{m}") for m in range(MT)]
            csum = [stats.tile([P, NST], f32, name=f"csum{m}") for m in range(MT)]
            ngm = [stats.tile([P, 1], f32, name=f"ngm{m}") for m in range(MT)]
            sfac = [stats.tile([P, NST], f32, name=f"sfac{m}") for m in range(MT)]
            wsum = [stats.tile([P, NST], f32, name=f"wsum{m}") for m in range(MT)]
            rsum6 = [stats.tile([P, 1], f32, name=f"rsum6{m}") for m in range(MT)]
            rsum = [stats.tile([P, 1], f32, name=f"rsum{m}") for m in range(MT)]
            rinv = [stats.tile([P, 1], f32, name=f"rinv{m}") for m in range(MT)]
            factor = [stats.tile([P, NST], f32, name=f"factor{m}") for m in range(MT)]

            COPY = mybir.ActivationFunctionType.Copy

            def epilogue_b(m):
                """Finish softmax stats for m-tile m (group 7b terms only),
                rescale split across DVE/ACT, store segments on both rings."""
                H = JCH // 2
                # sfac_7b on ACT right behind this m's group-7b exp
                nc.scalar.activation(
                    out=sfac[m][:, NST - 1 : NST],
                    in_=ncmax[m][:, NST - 1 : NST],
                    func=EXP,
                    bias=ngm[m],
                    scale=-1.0,
                )
                # If group 7b holds the row's global max by more than ~88,
                # exp overflows to inf and its factor becomes inf*0=NaN.
                # Clamping is exact in that regime: the factor -> 1/csum
                # and the other chunks' factors underflow to 0 as they
                # mathematically should.
                nc.vector.tensor_scalar_min(
                    sfac[m][:, NST - 1 : NST], sfac[m][:, NST - 1 : NST], 1.0e30
                )
                # rsum = csum_7b * sfac_7b + rsum(0..7a) in one DVE op
                nc.vector.scalar_tensor_tensor(
                    out=rsum[m],
                    in0=csum[m][:, NST - 1 : NST],
                    scalar=sfac[m][:, NST - 1 : NST],
                    in1=rsum6[m],
                    op0=mybir.AluOpType.mult,
                    op1=ADD,
                )
                nc.vector.reciprocal(out=rinv[m], in_=rsum[m])
                nc.vector.tensor_scalar_mul(factor[m], sfac[m], rinv[m])
                # rescale: DVE takes 7b then chunks 0-4 (~0.35us each at
                # 512 wide); ACT takes 7a then 5,6 via Copy with
                # per-partition scale (~0.8us each).  (GpSimd tensor_scalar
                # measured 7.5us/chunk and stalls concurrent DVE ops --
                # unusable.)  Stores are segments (a dma_start costs
                # ~0.6us of issuing-engine time): the freshly-finished
                # tail cols 3584:4096 go out first on SP, then {0-2},
                # {3,4} on SP while {5,6} rides the ACT ring.
                row = out[m * P : (m + 1) * P, :]
                c7 = (JT - 1) * JCH
                # DVE: group 7b first
                a_sl = a_tiles[m][:, c7 + H : c7 + 2 * H]
                nc.vector.tensor_scalar_mul(
                    a_sl, a_sl, factor[m][:, NST - 1 : NST]
                )
                # ACT: group 7a first, then chunks 5, 6
                a_sl = a_tiles[m][:, c7 : c7 + H]
                nc.scalar.activation(
                    out=a_sl, in_=a_sl, func=COPY, bias=0.0,
                    scale=factor[m][:, JT - 1 : JT],
                )
                # tail cols (both halves of old chunk 7) on SP ring
                nc.sync.dma_start(
                    out=row[:, c7 : c7 + 2 * H], in_=a_tiles[m][:, c7 : c7 + 2 * H]
                )
                for j2 in (0, 1, 2, 3, 4):
                    a_sl = a_tiles[m][:, j2 * JCH : (j2 + 1) * JCH]
                    nc.vector.tensor_scalar_mul(a_sl, a_sl, factor[m][:, j2 : j2 + 1])
                    if j2 == 2:
                        nc.sync.dma_start(
                            out=row[:, 0 : 3 * JCH], in_=a_tiles[m][:, 0 : 3 * JCH]
                        )
                    elif j2 == 4:
                        nc.sync.dma_start(
                            out=row[:, 3 * JCH : 5 * JCH],
                            in_=a_tiles[m][:, 3 * JCH : 5 * JCH],
                        )
                for j2 in (5, 6):
                    a_sl = a_tiles[m][:, j2 * JCH : (j2 + 1) * JCH]
                    nc.scalar.activation(
                        out=a_sl, in_=a_sl, func=COPY, bias=0.0,
                        scale=factor[m][:, j2 : j2 + 1],
                    )
                nc.scalar.dma_start(
                    out=row[:, 5 * JCH : 7 * JCH], in_=a_tiles[m][:, 5 * JCH : 7 * JCH]
                )

            def load_slab(j):
                slabs = []
                for g in range(DT // GSL):
                    wst_sl = wstpool.tile([P, GSL, JCH], f16, name="wst_sl")
                    if j < 2:
                        # write-before-write gate: orders the slab DMA after
                        # stage-1 q1/q2 so the key-slab prefetch doesn't
                        # steal HBM bandwidth from the W feed
                        nc.vector.tensor_copy(
                            out=wst_sl[:, 0, 0:1], in_=tT[:, 4 * (j + 1), 0:1]
                        )
                    nc.sync.dma_start(out=wst_sl, in_=wst[j, g])
                    slabs.append(wst_sl)
                return slabs

            def chunk_group(si, m, slabs, c0, width, s0):
                ps2 = psum.tile([P, width], f32, name="ps2", tag="ps")
                for d in range(DT):
                    nc.tensor.matmul(
                        ps2,
                        tT[:, d, m * P : (m + 1) * P],
                        slabs[d // GSL][:, d % GSL, s0 : s0 + width],
                        start=(d == 0),
                        stop=(d == DT - 1),
                    )
                # chunk softmax: -max, then exp(x - max) with running sum
                nc.vector.reduce_max(
                    out=ncmax[m][:, si : si + 1], in_=ps2, axis=X, negate=True
                )
                nc.scalar.activation(
                    out=a_tiles[m][:, c0 : c0 + width],
                    in_=ps2,
                    func=EXP,
                    bias=ncmax[m][:, si : si + 1],
                    scale=1.0,
                    accum_out=csum[m][:, si : si + 1],
                )

            def part_a(m):
                # stats over chunk groups 0..7a, run in the shadow of the
                # final matmul groups.  The softmax factor is invariant to
                # the shift S, so S = max over the first 8 groups works;
                # only group 7b's contribution is added late in epilogue_b.
                nc.vector.tensor_reduce(
                    out=ngm[m], in_=ncmax[m][:, 0 : NST - 1], axis=X, op=MIN
                )
                nc.scalar.activation(
                    out=sfac[m][:, 0 : NST - 1],
                    in_=ncmax[m][:, 0 : NST - 1],
                    func=EXP,
                    bias=ngm[m],
                    scale=-1.0,
                )
                nc.vector.tensor_mul(
                    out=wsum[m][:, 0 : NST - 1],
                    in0=sfac[m][:, 0 : NST - 1],
                    in1=csum[m][:, 0 : NST - 1],
                )
                nc.vector.tensor_reduce(
                    out=rsum6[m], in_=wsum[m][:, 0 : NST - 1], axis=X, op=ADD
                )

            for si, c0, width in CHUNKS[: JT - 2]:
                slabs = load_slab(si)
                for m in range(MT):
                    chunk_group(si, m, slabs, c0, width, 0)

            # The final three chunk groups (j6, 7a, 7b) are interleaved
            # per m-tile so each m-tile's softmax epilogue gets a ~6.8us
            # shadow window under the remaining matmul groups, and the
            # last m-tile's epilogue chain starts right at the final
            # (half-width) matmul group.
            slabs6 = load_slab(JT - 2)
            slabs7 = load_slab(JT - 1)
            for m in range(MT):
                chunk_group(JT - 2, m, slabs6, (JT - 2) * JCH, JCH, 0)
                chunk_group(JT - 1, m, slabs7, (JT - 1) * JCH, JCH // 2, 0)
                part_a(m)
                chunk_group(
                    JT, m, slabs7, (JT - 1) * JCH + JCH // 2, JCH // 2, JCH // 2
                )
                epilogue_b(m)

    nc.compile()
    return nc


def get_nc():
    global _NC_CACHE
    if _NC_CACHE is None:
        _NC_CACHE = _build_nc()
    return _NC_CACHE


def make_in_maps(Ws, W):
    Ws = np.asarray(Ws, dtype=np.float32)
    W0 = np.asarray(W, dtype=np.float32).reshape(D, D)
    # W pre-tile: [q, kk, p, ki, c] so each [128, 2, 512] chunk is a
    # contiguous 2 KB/partition read
    w_t = np.ascontiguousarray(
        W0.reshape(WKK, 2, P, NQ, QW).transpose(3, 0, 2, 1, 4)
    ).astype(np.float16)
    # Ws.T pre-tile: [j, g, p, ti, c] so each [128, 4, 512] slab is a
    # contiguous 4 KB/partition read
    WsT = np.ascontiguousarray(Ws.T)  # [D, N]
    wst_t = np.ascontiguousarray(
        WsT.reshape(DT // GSL, GSL, P, JT, JCH).transpose(3, 0, 2, 1, 4)
    ).astype(np.float16)
    in_maps = []
    for c in range(NCORES):
        shard_t = np.ascontiguousarray(
            Ws[c * M : (c + 1) * M, :].T.reshape(KT, P, M).transpose(1, 0, 2)
        ).astype(np.float16)
        in_maps.append({"wsT_shard": shard_t, "w_mat": w_t, "wsT_full": wst_t})
    return in_maps


def _run_device(in_maps):
    from concourse.bass_utils import run_bass_kernel_spmd

    nc = get_nc()
    res = run_bass_kernel_spmd(nc, in_maps, core_ids=list(range(NCORES)))
    return np.concatenate(
        [res.results[c]["out"] for c in range(NCORES)], axis=0
    )


def kernel(Ws, W, b, **_unused):
    # b[0] is a constant additive shift on every score; softmax over
    # axis=1 is invariant to it, so it never enters the device kernel.
    in_maps = make_in_maps(Ws, W)
    try:
        out = _run_device(in_maps)
    except Exception as e:  # transient device failures recover on retry
        import sys, traceback

        traceback.print_exc()
        print(f"device run failed ({e!r}); retrying once", file=sys.stderr)
        try:
            out = _run_device(in_maps)
        except Exception:
            traceback.print_exc()
            print("device retry failed; numpy fallback", file=sys.stderr)
            Wsf = np.asarray(Ws, dtype=np.float32)
            A = (Wsf @ np.asarray(W, np.float32).reshape(D, D)) @ Wsf.T
            A += np.asarray(b, np.float32).reshape(-1)[0]
            A -= A.max(axis=1, keepdims=True)
            np.exp(A, out=A)
            A /= A.sum(axis=1, keepdims=True)
            return A
    return np.ascontiguousarray(out.astype(np.float32))


if __name__ == "__main__":
    rng = np.random.default_rng(0)
    Ws = rng.standard_normal((N, D), dtype=np.float32)
    W = (rng.standard_normal((1, D, D)) / np.sqrt(D)).astype(np.float32)
    b = np.zeros((1,), dtype=np.float32)
    res = kernel(Ws=Ws, W=W, b=b)
    print(res.shape, res.dtype, res.sum())
